# revision 9
# baseline (speedup 1.0000x reference)
"""GCN (2-layer, PyG GCNConv-style) on 8 Trainium2 NeuronCores.

Strategy (sharding_hint): nodes sharded across the 8 cores (data parallel on
the node dim); edges partitioned by destination core so the scatter-add stays
local; per layer the dinv-scaled transformed features are AllGathered so each
core can gather arbitrary source rows; weights replicated.

Math (per layer, A' = A + I, dinv = deg^-1/2):
    out = dinv . (A'^T (dinv . (x @ W))) + b
We fold norms so no per-edge scaling is needed:
  - table  = dinv . (x @ W)                    (per-node scale, ACT)
  - agg    = A'^T table  + b * sqrtdeg         (PE one-hot matmuls)
  - layer1 h2 = dinv . relu(agg1)              (dinv moved past relu, dinv>0)
    the dinv is then folded into layer2's table scale (dinv^2).
  - layer2 out = dinv . agg2                   (final per-node scale)

Edge aggregation on device: edges are grouped on host by (dst-tile t of 128
nodes, src-group g of 32768 nodes), each (g,t) segment padded to a multiple
of 128.  Source rows are fetched with dma_gather (256B rows of the gathered
fp16 table), and the scatter-add is a PE matmul with a one-hot selection
matrix built on the vector engine:  psum[f, d] += sum_e msgs[e, f] *
(dstloc[e] == d).
"""

import functools
import numpy as np

import concourse.bacc as bacc
import concourse.mybir as mybir
import concourse.tile as tile
from concourse.bass_utils import run_bass_kernel_spmd
from concourse.masks import make_identity

NCORE = 8
P = 128
GSHIFT = 15  # src-group size 32768 (int16 index range)
GSZ = 1 << GSHIFT
TBSZ = 8  # dst tiles per gather block
SINGLE_PACKET = False  # experiment: SWDGE packet mode
APG_PROBE = 3  # throwaway ap_gather probes for rate measurement (0=off)

F16 = mybir.dt.float16
F32 = mybir.dt.float32
I16 = mybir.dt.int16


def _round_up(a, b):
    return (a + b - 1) // b * b


# ----------------------------------------------------------------------------
# Bass program (identical for all 8 cores; per-core data differs via inputs)
# ----------------------------------------------------------------------------

_PHASES = 6  # debug: stop building after this phase (1..6)
_EMODE = "full"  # debug: edge-phase content: gather | ind | mm_b | full


def _build(cfg):
    """cfg = (F, H, C, NS, ecnt) with ecnt[g][t] = padded edge count."""
    Fdim, H, C, NS, ecnt = cfg
    PH = _PHASES
    EM = _EMODE
    ecnt = [list(gr) for gr in ecnt]
    T = NS // P
    NPAD = NCORE * NS
    NG = len(ecnt)
    E_TOT = sum(sum(gr) for gr in ecnt)
    blocks = [list(range(b, min(b + TBSZ, T))) for b in range(0, T, TBSZ)]
    # flat edge order: for block: for g: for t in block: ecnt[g][t] edges
    SCMAX = max(
        sum(ecnt[g][t] for t in blk) // P for blk in blocks for g in range(NG)
    )

    nc = bacc.Bacc(None, target_bir_lowering=False)

    # ---- I/O ----
    xT_in = nc.dram_tensor("xT", [P, NS], F32, kind="ExternalInput")
    degnm_in = nc.dram_tensor("deg_nm", [P, T], F32, kind="ExternalInput")
    degrow_in = nc.dram_tensor("deg_row", [1, NS], F32, kind="ExternalInput")
    w1_in = nc.dram_tensor("W1", [Fdim, H], F32, kind="ExternalInput")
    w2_in = nc.dram_tensor("W2", [H, C], F32, kind="ExternalInput")
    b1_in = nc.dram_tensor("b1", [1, H], F32, kind="ExternalInput")
    b2_in = nc.dram_tensor("b2", [1, C], F32, kind="ExternalInput")
    gidx_in = nc.dram_tensor("gidx", [P, E_TOT // 16], I16, kind="ExternalInput")
    apgidx_in = nc.dram_tensor("apgidx", [P, 128], I16, kind="ExternalInput")
    apgdbg_out = nc.dram_tensor("apg_dbg", [P, 4096], F16, kind="ExternalOutput")
    dloc_in = nc.dram_tensor("dloc", [P, E_TOT // P], F16, kind="ExternalInput")
    out_ext = nc.dram_tensor("out_nm", [NS, C], F32, kind="ExternalOutput")

    hsh = nc.dram_tensor("hsh", [NS, P], F16)
    gsh = nc.dram_tensor("gsh", [NS, P], F16)
    hfull = nc.dram_tensor("hfull", [NPAD, P], F16, addr_space="Shared")
    gfull = nc.dram_tensor("gfull", [NPAD, P], F16, addr_space="Shared")
    rgroups = [list(range(NCORE))]

    with tile.TileContext(nc) as tc:
        with (
            tc.tile_pool(name="con", bufs=1) as con,
            tc.tile_pool(name="meta", bufs=1) as meta,
            tc.tile_pool(name="stg", bufs=1) as stg,
            tc.tile_pool(name="io", bufs=3) as io,
            tc.tile_pool(name="eb", bufs=2) as eb,
            tc.tile_pool(name="acc", bufs=2) as acc_pool,
            tc.tile_pool(name="ps", bufs=3, space="PSUM") as ps,
            tc.tile_pool(name="pst", bufs=2, space="PSUM") as pst,
        ):
            # ---- Phase A: constants / metadata ----
            dloc = meta.tile([P, E_TOT // P], F16)
            nc.sync.dma_start(dloc[:], dloc_in[:])

            w1f = con.tile([Fdim, H], F32)
            nc.sync.dma_start(w1f[:], w1_in[:])
            w1 = con.tile([Fdim, H], F16)
            nc.vector.tensor_copy(w1[:], w1f[:])
            w2f = con.tile([H, C], F32)
            nc.sync.dma_start(w2f[:], w2_in[:])
            w2 = con.tile([H, C], F16)
            nc.vector.tensor_copy(w2[:], w2f[:])
            b1f = con.tile([1, H], F32)
            nc.sync.dma_start(b1f[:], b1_in[:])
            b1 = con.tile([1, H], F16)
            nc.vector.tensor_copy(b1[:], b1f[:])
            b2f = con.tile([1, C], F32)
            nc.sync.dma_start(b2f[:], b2_in[:])
            b2 = con.tile([1, C], F16)
            nc.vector.tensor_copy(b2[:], b2f[:])

            degnm = con.tile([P, T], F32)
            nc.sync.dma_start(degnm[:], degnm_in[:])
            sq_nm = con.tile([P, T], F32)
            nc.scalar.activation(sq_nm[:], degnm[:], mybir.ActivationFunctionType.Sqrt)
            dinv_nm = con.tile([P, T], F32)
            nc.vector.reciprocal(dinv_nm[:], sq_nm[:])
            dinv2_nm = con.tile([P, T], F32)
            nc.vector.tensor_mul(dinv2_nm[:], dinv_nm[:], dinv_nm[:])

            degrow = con.tile([1, NS], F32)
            nc.sync.dma_start(degrow[:], degrow_in[:])
            sqrow = con.tile([1, NS], F16)
            nc.scalar.activation(sqrow[:], degrow[:], mybir.ActivationFunctionType.Sqrt)

            iota_i = con.tile([P, P], I16)
            nc.gpsimd.iota(iota_i[:], pattern=[[1, P]], base=0, channel_multiplier=0)
            iota16 = con.tile([P, P], F16)
            nc.vector.tensor_copy(iota16[:], iota_i[:])

            ident = con.tile([P, P], F32)
            make_identity(nc, ident[:])

            stage = stg.tile([P, T, P], F16, tag="stage")
            nc.vector.memset(stage[:], 0.0)

            # ---- Phase B: layer-1 transform, build h' table ----
            for t in range(T):
                xt = io.tile([P, P], F32, tag="xt")
                nc.sync.dma_start(xt[:], xT_in[:, t * P : (t + 1) * P])
                xt16 = io.tile([P, P], F16, tag="xt16")
                nc.vector.tensor_copy(xt16[:], xt[:])
                ph = pst.tile([P, H], F32, tag="pt")
                nc.tensor.matmul(ph[:], xt16[:], w1[:], start=True, stop=True)
                nc.scalar.activation(
                    stage[:, t, 0:H],
                    ph[:],
                    mybir.ActivationFunctionType.Copy,
                    scale=dinv_nm[:, t : t + 1],
                )
            nc.sync.dma_start(hsh.rearrange("(t p) d -> p t d", p=P)[:], stage[:])

            # throwaway ap_gather probes: gather 16384 4B-cells from a dummy
            # [128, 8192, 2] fp16 view of the stage buffer; idx = iota%8192
            if APG_PROBE:
                apg_idx = con.tile([P, 128], I16)
                nc.sync.dma_start(apg_idx[:], apgidx_in[:])
                apg_src = dloc.rearrange("p (n d) -> p n d", d=2)[:, 0:768, :]
                apg_out = con.tile([P, 4096], F16, tag="apgprobe")
                for _r in range(APG_PROBE):
                    nc.gpsimd.ap_gather(
                        apg_out.rearrange("p (n d) -> p n d", d=2)[:],
                        apg_src,
                        apg_idx[:, 0:128],
                        128, 768, 2, 2048,
                    )
                nc.sync.dma_start(apgdbg_out[:], apg_out[:])

            # ---- Phase C: AllGather layer-1 table ----
            if PH >= 2:
                nc.gpsimd.collective_compute(
                    "AllGather",
                    mybir.AluOpType.bypass,
                    ins=[hsh[:]],
                    outs=[hfull[:]],
                    replica_groups=rgroups,
                )

            rt16 = stg.tile([H, T * P], F16)

            def edge_phase(table, width, bvec, accw, layer):
                """Aggregate edges: acc_blocks[t][f, d] = (A'^T msgs)[d, f] + b*sqrtdeg."""
                off = 0
                out_blocks = []
                for blk in blocks:
                    accb = acc_pool.tile([accw, TBSZ * P], F32, tag=f"acc{layer}")
                    for g in range(NG):
                        B = sum(ecnt[g][t] for t in blk)
                        if B > 0 and EM != "none":
                            sc = B // P
                            msgs = eb.tile([P, SCMAX, P], F16, tag="msgs")
                            gbase = g * GSZ
                            gsz = min(GSZ, NPAD - gbase)
                            gi = eb.tile([P, SCMAX * 8], I16, tag="gi")
                            nc.sync.dma_start(
                                gi[:, 0 : B // 16],
                                gidx_in[:, off // 16 : (off + B) // 16],
                            )
                            nc.gpsimd.dma_gather(
                                msgs[:, 0:sc, :],
                                table[gbase : gbase + gsz, :],
                                gi[:, 0 : B // 16],
                                B,
                                B,
                                P,
                                single_packet=SINGLE_PACKET,
                            )
                            ind = eb.tile([P, SCMAX, P], F16, tag="ind")
                            if EM in ("ind", "mm_b", "full"):
                                nc.vector.tensor_tensor(
                                    out=ind[:, 0:sc, :],
                                    in0=iota16[:, :].rearrange("p (s d) -> p s d", s=1).to_broadcast([P, sc, P]),
                                    in1=dloc[:, off // P : off // P + sc].rearrange("p (s o) -> p s o", o=1).to_broadcast([P, sc, P]),
                                    op=mybir.AluOpType.is_equal,
                                )
                        si = 0
                        for ti, t in enumerate(blk):
                            nch = ecnt[g][t] // P if EM == "full" else 0
                            emit_b = g == 0 and EM in ("mm_b", "full")
                            pa = ps.tile([accw, P], F32, tag="pa")
                            nmm = 0
                            if emit_b:
                                nc.tensor.matmul(
                                    pa[:],
                                    bvec[:],
                                    sqrow[0:1, t * P : (t + 1) * P],
                                    start=True,
                                    stop=(nch == 0),
                                )
                                nmm = 1
                            for k in range(nch):
                                nc.tensor.matmul(
                                    pa[:],
                                    msgs[:, si + k, 0:width],
                                    ind[:, si + k, :],
                                    start=(nmm == 0 and k == 0),
                                    stop=(k == nch - 1),
                                )
                            si += nch
                            if nmm + nch == 0:
                                pass
                            elif g == 0:
                                nc.vector.tensor_copy(accb[:, ti * P : (ti + 1) * P], pa[:])
                            elif nch > 0:
                                nc.vector.tensor_add(
                                    out=accb[:, ti * P : (ti + 1) * P],
                                    in0=accb[:, ti * P : (ti + 1) * P],
                                    in1=pa[:],
                                )
                        off += B
                    if EM != "full":
                        nc.vector.memset(accb[:], 0.0)
                    out_blocks.append((blk, accb))
                return out_blocks

            # ---- Phase D: layer-1 edge aggregation + relu + layer-2 transform ----
            for blk, accb in (edge_phase(hfull, H, b1, H, 1) if PH >= 3 else []):
                for ti, t in enumerate(blk):
                    nc.scalar.activation(
                        rt16[:, t * P : (t + 1) * P],
                        accb[:, ti * P : (ti + 1) * P],
                        mybir.ActivationFunctionType.Relu,
                    )
            if PH >= 4:
                # stage buffer reused for the layer-2 table (pad cols must be 0)
                nc.vector.memset(stage[:], 0.0)
                for t in range(T):
                    pg = pst.tile([P, C], F32, tag="pt")
                    nc.tensor.matmul(
                        pg[:], rt16[:, t * P : (t + 1) * P], w2[:], start=True, stop=True
                    )
                    nc.scalar.activation(
                        stage[:, t, 0:C],
                        pg[:],
                        mybir.ActivationFunctionType.Copy,
                        scale=dinv2_nm[:, t : t + 1],
                    )
                nc.sync.dma_start(gsh.rearrange("(t p) d -> p t d", p=P)[:], stage[:])

            # ---- Phase E: AllGather layer-2 table ----
            if PH >= 5:
                nc.gpsimd.collective_compute(
                    "AllGather",
                    mybir.AluOpType.bypass,
                    ins=[gsh[:]],
                    outs=[gfull[:]],
                    replica_groups=rgroups,
                )

            # ---- Phase F: layer-2 edge aggregation + final scale ----
            out_stage = stg.tile([P, T, C], F32, tag="stage")
            nc.vector.memset(out_stage[:], 0.0)
            for blk, accb in (edge_phase(gfull, C, b2, C, 2) if PH >= 6 else []):
                for ti, t in enumerate(blk):
                    ptr = pst.tile([P, C], F32, tag="pt")
                    nc.tensor.transpose(
                        out=ptr[:],
                        in_=accb[:, ti * P : (ti + 1) * P],
                        identity=ident[0:C, 0:C],
                    )
                    nc.scalar.activation(
                        out_stage[:, t, :],
                        ptr[:],
                        mybir.ActivationFunctionType.Copy,
                        scale=dinv_nm[:, t : t + 1],
                    )
            nc.sync.dma_start(out_ext.rearrange("(t p) c -> p t c", p=P)[:], out_stage[:])

    nc.compile()
    return nc


def _build_cached(cfg_key):
    return _build_cached_ph(cfg_key, _PHASES, _EMODE)


@functools.lru_cache(maxsize=8)
def _build_cached_ph(cfg_key, ph, em):
    global _PHASES, _EMODE
    _PHASES = ph
    _EMODE = em
    Fdim, H, C, NS, ecnt_t = cfg_key
    return _build((Fdim, H, C, NS, [list(g) for g in ecnt_t]))


# ----------------------------------------------------------------------------
# Host-side sharding / metadata prep
# ----------------------------------------------------------------------------

def _prep(x, edge_index, W1, b1, W2, b2):
    N, Fdim = x.shape
    H = W1.shape[1]
    C = W2.shape[1]
    NS = _round_up(-(-N // NCORE), P)
    T = NS // P
    NPAD = NCORE * NS
    NG = -(-NPAD // GSZ)

    src = np.asarray(edge_index[0], dtype=np.int64)
    dst = np.asarray(edge_index[1], dtype=np.int64)
    # self loops
    loops = np.arange(N, dtype=np.int64)
    src = np.concatenate([src, loops])
    dst = np.concatenate([dst, loops])

    deg = np.bincount(dst, minlength=N).astype(np.float32)  # includes self loop? no:
    # reference: deg = segment_sum(ones over all edges incl self loops) -> bincount of
    # the concatenated dst already includes the self loops.
    deg_pad = np.ones(NPAD, dtype=np.float32)
    deg_pad[:N] = deg

    core = dst // NS
    t_of = (dst % NS) >> 7
    g_of = src >> GSHIFT
    d_of = dst & (P - 1)

    # per (core, g, t) counts -> shared padded counts ecnt[g][t]
    seg_id = (core * NG + g_of) * T + t_of
    cnt = np.bincount(seg_id, minlength=NCORE * NG * T).reshape(NCORE, NG, T)
    ecnt = _round_up(cnt.max(axis=0), P)  # [NG, T] shared across cores
    ecnt[ecnt == 0] = 0

    blocks = [list(range(b, min(b + TBSZ, T))) for b in range(0, T, TBSZ)]

    # flat offsets in the (block, g, t) stream
    flat_off = np.zeros((NG, T), dtype=np.int64)
    off = 0
    for blk in blocks:
        for g in range(NG):
            for t in blk:
                flat_off[g, t] = off
                off += ecnt[g, t]
    E_TOT = off

    # position of each edge inside its (core,g,t) segment
    order = np.argsort(seg_id, kind="stable")
    seg_sorted = seg_id[order]
    starts = np.searchsorted(seg_sorted, np.arange(NCORE * NG * T))
    rank = np.arange(len(order)) - starts[seg_sorted]
    pos_sorted = flat_off[(seg_sorted // T) % NG, seg_sorted % T] + rank
    core_sorted = seg_sorted // (NG * T)

    gidx_all = np.zeros((NCORE, E_TOT), dtype=np.int16)
    dloc_all = np.full((NCORE, E_TOT), -1.0, dtype=np.float16)
    gidx_all[core_sorted, pos_sorted] = (src[order] - (g_of[order] << GSHIFT)).astype(
        np.int16
    )
    dloc_all[core_sorted, pos_sorted] = d_of[order].astype(np.float16)

    x_pad = np.zeros((NPAD, Fdim), dtype=np.float32)
    x_pad[:N] = np.asarray(x, dtype=np.float32)

    rng_apg = np.random.default_rng(7)
    apg_idxs = rng_apg.integers(0, 768, size=(2048,)).astype(np.int16)
    apgidx_w = np.tile(np.ascontiguousarray(apg_idxs.reshape(128, 16).T), (NCORE, 1))

    in_maps = []
    for c in range(NCORE):
        xT = np.ascontiguousarray(x_pad[c * NS : (c + 1) * NS].T)
        dshard = deg_pad[c * NS : (c + 1) * NS]
        deg_nm = np.ascontiguousarray(dshard.reshape(T, P).T)
        deg_row = dshard.reshape(1, NS)
        flat = gidx_all[c]
        gidx_w = np.tile(
            np.ascontiguousarray(flat.reshape(E_TOT // 16, 16).T), (NCORE, 1)
        )
        dloc_w = np.ascontiguousarray(dloc_all[c].reshape(E_TOT // P, P).T)
        in_maps.append(
            {
                "xT": xT,
                "deg_nm": deg_nm,
                "deg_row": deg_row,
                "W1": np.asarray(W1, dtype=np.float32).reshape(Fdim, H),
                "W2": np.asarray(W2, dtype=np.float32).reshape(H, C),
                "b1": np.asarray(b1, dtype=np.float32).reshape(1, H),
                "b2": np.asarray(b2, dtype=np.float32).reshape(1, C),
                "gidx": gidx_w,
                "dloc": dloc_w,
                "apgidx": apgidx_w,
            }
        )

    cfg_key = (Fdim, H, C, NS, tuple(tuple(int(v) for v in row) for row in ecnt))
    return cfg_key, in_maps, N, NS, C


apg_idxs_global = np.random.default_rng(7).integers(0, 768, size=(2048,)).astype(np.int16)


def _run(x, edge_index, W1, b1, W2, b2, trace=False):
    cfg_key, in_maps, N, NS, C = _prep(x, edge_index, W1, b1, W2, b2)
    nc = _build_cached(cfg_key)
    res = run_bass_kernel_spmd(nc, in_maps, list(range(NCORE)), trace=trace)
    if "apg_dbg" in res.results[0]:
        dbg = res.results[0]["apg_dbg"]
        dl = in_maps[0]["dloc"]
        idxs = np.tile(apg_idxs_global.reshape(128, 16).T, (NCORE, 1))[0:16, :]
        # expected: out[p, i, :] = src[p, idx_i, :], idx stream wraps 16 partitions
        src = dl[:, 0 : 768 * 2].reshape(128, 768, 2)
        flat_idx = apg_idxs_global
        exp = np.zeros((128, 2048, 2), np.float16)
        for i in range(2048):
            exp[:, i, :] = src[:, flat_idx[i], :]
        got = dbg.reshape(128, 2048, 2)
        err = np.abs(exp.astype(np.float32) - got.astype(np.float32)).max()
        print(f"APG_DBG max err: {err}")
    shards = [res.results[c]["out_nm"] for c in range(NCORE)]
    out = np.concatenate(shards, axis=0)[:N]
    return np.ascontiguousarray(out, dtype=np.float32), res


def kernel(x, edge_index, W1, b1, W2, b2):
    out, _ = _run(x, edge_index, W1, b1, W2, b2)
    return out



# revision 18
# speedup vs baseline: 1.8301x; 1.8301x over previous
"""GCN (2-layer, PyG GCNConv-style) on 8 Trainium2 NeuronCores.

Strategy (sharding_hint): nodes sharded across the 8 cores (data parallel on
the node dim); edges partitioned by destination core so the scatter-add stays
local; per layer the dinv-scaled transformed features are AllGathered so each
core can gather arbitrary source rows; weights replicated.

Math (per layer, A' = A + I, dinv = deg^-1/2):
    out = dinv . (A'^T (dinv . (x @ W))) + b
We fold norms so no per-edge scaling is needed:
  - table  = dinv . (x @ W)                    (per-node scale, ACT)
  - agg    = A^T table + table + b * sqrtdeg   (PE one-hot matmuls; the
             self-loop term "+ table" is a dense per-tile transpose matmul,
             so self loops never enter the gathered edge streams)
  - layer1 h2 = dinv . relu(agg)               (dinv moved past relu, dinv>0)
    the dinv is then folded into layer2's table scale (dinv^2).
  - layer2 out = dinv . agg2                   (final per-node scale)

Edge aggregation: edges are grouped on host by (dst-tile t of 128 nodes,
src-group g of 32768 nodes); per (g,t) the segment is padded to a multiple of
16 only (pad idx=0, pad dst label -1).  Segments are concatenated per
(g, tile-block) into one gather call; matmul chunks of 128 gathered rows may
straddle segment boundaries, which is handled with partition-sliced one-hot
matmuls (the straddle map is static because segment sizes are shared across
cores; per-core count variation hides in the dst labels, where -1 kills the
one-hot).  Scatter-add accumulates directly into one PSUM tile per dst tile.
"""

import functools
import numpy as np

import concourse.bacc as bacc
import concourse.mybir as mybir
import concourse.tile as tile
from concourse.bass_utils import run_bass_kernel_spmd
from concourse.masks import make_identity

NCORE = 8
P = 128
GSHIFT = 15  # src-group size 32768 (int16 index range)
GSZ = 1 << GSHIFT
TBSZ = 4  # dst tiles per gather block

F16 = mybir.dt.float16
F32 = mybir.dt.float32
I16 = mybir.dt.int16


def _round_up(a, b):
    return (a + b - 1) // b * b


def _call_layout(ecnt, blocks, NG):
    """Static stream layout.  Returns per-call info:
    calls[(bi, g)] = dict(B, off16, offch, cov) where cov[k] = list of
    (t, lo, hi) partition-slices of chunk k."""
    calls = {}
    off16 = 0  # global gidx offset, units of 16 idxs
    offch = 0  # global chunk offset (dloc columns)
    for bi, blk in enumerate(blocks):
        for g in range(NG):
            segs = []
            s = 0
            for t in blk:
                e = ecnt[g][t]
                segs.append((t, s, e))
                s += e
            B = s
            K = -(-B // P) if B else 0
            cov = []
            for k in range(K):
                lo_k, hi_k = k * P, (k + 1) * P
                entries = []
                for t, st, e in segs:
                    if e == 0:
                        continue
                    a, b = max(st, lo_k), min(st + e, hi_k)
                    if a < b:
                        entries.append([t, a - lo_k, b - lo_k])
                cov.append(entries)
            calls[(bi, g)] = dict(B=B, off16=off16, offch=offch, cov=cov)
            off16 += B // 16
            offch += K
    return calls, off16 * 16, offch


# ----------------------------------------------------------------------------
# Bass program (identical for all 8 cores; per-core data differs via inputs)
# ----------------------------------------------------------------------------


def _build(cfg):
    """cfg = (F, H, C, NS, ecnt) with ecnt[g][t] = padded16 edge count."""
    Fdim, H, C, NS, ecnt = cfg
    ecnt = [list(gr) for gr in ecnt]
    T = NS // P
    NPAD = NCORE * NS
    NG = len(ecnt)
    blocks = [list(range(b, min(b + TBSZ, T))) for b in range(0, T, TBSZ)]
    calls, E_IDX, NCH = _call_layout(ecnt, blocks, NG)
    SCMAX = max(len(c["cov"]) for c in calls.values())

    nc = bacc.Bacc(None, target_bir_lowering=False)

    # ---- I/O ----
    xT_in = nc.dram_tensor("xT", [P, NS], F32, kind="ExternalInput")
    degnm_in = nc.dram_tensor("deg_nm", [P, T], F32, kind="ExternalInput")
    degrow_in = nc.dram_tensor("deg_row", [1, NS], F32, kind="ExternalInput")
    w1_in = nc.dram_tensor("W1", [Fdim, H], F32, kind="ExternalInput")
    w2_in = nc.dram_tensor("W2", [H, C], F32, kind="ExternalInput")
    b1_in = nc.dram_tensor("b1", [1, H], F32, kind="ExternalInput")
    b2_in = nc.dram_tensor("b2", [1, C], F32, kind="ExternalInput")
    gidx_in = nc.dram_tensor("gidx", [P, E_IDX // 16], I16, kind="ExternalInput")
    dloc_in = nc.dram_tensor("dloc", [P, NCH], F16, kind="ExternalInput")
    out_ext = nc.dram_tensor("out_nm", [NS, C], F32, kind="ExternalOutput")

    hsh = nc.dram_tensor("hsh", [NS, P], F16)
    gsh = nc.dram_tensor("gsh", [NS, P], F16)
    hfull = nc.dram_tensor("hfull", [NPAD, P], F16, addr_space="Shared")
    gfull = nc.dram_tensor("gfull", [NPAD, P], F16, addr_space="Shared")
    rgroups = [list(range(NCORE))]

    with tile.TileContext(nc) as tc:
        with (
            tc.tile_pool(name="con", bufs=1) as con,
            tc.tile_pool(name="meta", bufs=1) as meta,
            tc.tile_pool(name="stg", bufs=1) as stg,
            tc.tile_pool(name="io", bufs=3) as io,
            tc.tile_pool(name="eb", bufs=3) as eb,
            tc.tile_pool(name="ps", bufs=1, space="PSUM") as ps,
            tc.tile_pool(name="pst", bufs=2, space="PSUM") as pst,
            tc.tile_pool(name="pst2", bufs=2, space="PSUM") as pst2,
        ):
            # ---- Phase A: constants / metadata ----
            dloc = meta.tile([P, NCH], F16)
            nc.sync.dma_start(dloc[:], dloc_in[:])

            w1f = con.tile([Fdim, H], F32)
            nc.sync.dma_start(w1f[:], w1_in[:])
            w1 = con.tile([Fdim, H], F16)
            nc.vector.tensor_copy(w1[:], w1f[:])
            w2f = con.tile([H, C], F32)
            nc.sync.dma_start(w2f[:], w2_in[:])
            w2 = con.tile([H, C], F16)
            nc.vector.tensor_copy(w2[:], w2f[:])
            b1f = con.tile([1, H], F32)
            nc.sync.dma_start(b1f[:], b1_in[:])
            b1 = con.tile([1, H], F16)
            nc.vector.tensor_copy(b1[:], b1f[:])
            b2f = con.tile([1, C], F32)
            nc.sync.dma_start(b2f[:], b2_in[:])
            b2 = con.tile([1, C], F16)
            nc.vector.tensor_copy(b2[:], b2f[:])

            degnm = con.tile([P, T], F32)
            nc.sync.dma_start(degnm[:], degnm_in[:])
            sq_nm = con.tile([P, T], F32)
            nc.scalar.activation(sq_nm[:], degnm[:], mybir.ActivationFunctionType.Sqrt)
            dinv_nm = con.tile([P, T], F32)
            nc.vector.reciprocal(dinv_nm[:], sq_nm[:])
            dinv2_nm = con.tile([P, T], F32)
            nc.vector.tensor_mul(dinv2_nm[:], dinv_nm[:], dinv_nm[:])

            degrow = con.tile([1, NS], F32)
            nc.sync.dma_start(degrow[:], degrow_in[:])
            sqrow = con.tile([1, NS], F16)
            nc.scalar.activation(sqrow[:], degrow[:], mybir.ActivationFunctionType.Sqrt)

            iota_i = con.tile([P, P], I16)
            nc.gpsimd.iota(iota_i[:], pattern=[[1, P]], base=0, channel_multiplier=0)
            iota16 = con.tile([P, P], F16)
            nc.vector.tensor_copy(iota16[:], iota_i[:])

            ident = con.tile([P, P], F32)
            make_identity(nc, ident[:])
            ident16 = con.tile([P, P], F16)
            nc.vector.tensor_copy(ident16[:], ident[:])

            stage = stg.tile([P, T, H], F16, tag="stage")
            stage2 = stg.tile([P, T, C], F16, tag="stage2")

            # ---- Phase B: layer-1 transform, build h' table ----
            for t in range(T):
                xt = io.tile([P, P], F32, tag="xt")
                nc.sync.dma_start(xt[:], xT_in[:, t * P : (t + 1) * P])
                xt16 = io.tile([P, P], F16, tag="xt16")
                nc.vector.tensor_copy(xt16[:], xt[:])
                ph = pst.tile([P, H], F32, tag="pt")
                nc.tensor.matmul(ph[:], xt16[:], w1[:], start=True, stop=True)
                nc.scalar.activation(
                    stage[:, t, 0:H],
                    ph[:],
                    mybir.ActivationFunctionType.Copy,
                    scale=dinv_nm[:, t : t + 1],
                )
            nc.sync.dma_start(
                hsh.rearrange("(t p) d -> p t d", p=P)[:, :, 0:H], stage[:]
            )

            # ---- Phase C: AllGather layer-1 table ----
            nc.gpsimd.collective_compute(
                "AllGather",
                mybir.AluOpType.bypass,
                ins=[hsh[:]],
                outs=[hfull[:]],
                replica_groups=rgroups,
            )

            rt16 = stg.tile([H, T * P], F16)

            def edge_phase(table, width, bvec, accw, sstage, evac):
                """psum_t = b*sqrtdeg + table_t^T + A^T table, per dst tile.

                evac(t, psum_tile) consumes the finished accumulation."""
                for bi, blk in enumerate(blocks):
                    psums = {}
                    mm_total = {}
                    mm_done = {}
                    for t in blk:
                        mm_total[t] = 2  # bias + self-loop
                    for g in range(NG):
                        for ent in calls[(bi, g)]["cov"]:
                            for t, lo, hi in ent:
                                mm_total[t] += 1

                    def acc_mm(t, lhsT, rhs):
                        pa = psums[t]
                        k = mm_done[t]
                        nc.tensor.matmul(
                            pa, lhsT, rhs,
                            start=(k == 0), stop=(k == mm_total[t] - 1),
                        )
                        mm_done[t] = k + 1

                    for ti, t in enumerate(blk):
                        pa_t = ps.tile(
                            [accw, P], F32, name=f"acc{ti}", tag=f"acc{ti}"
                        )
                        psums[t] = pa_t[:]
                        mm_done[t] = 0
                        # bias * sqrtdeg row
                        acc_mm(t, bvec[:], sqrow[0:1, t * P : (t + 1) * P])
                        # dense self-loop: psum += table_t^T
                        acc_mm(t, sstage[:, t, 0:width], ident16[:, :])

                    for g in range(NG):
                        info = calls[(bi, g)]
                        B = info["B"]
                        if B == 0:
                            continue
                        sc = len(info["cov"])
                        gbase = g * GSZ
                        gsz = min(GSZ, NPAD - gbase)
                        gi = eb.tile([P, SCMAX * 8], I16, tag="gi")
                        nc.sync.dma_start(
                            gi[:, 0 : B // 16],
                            gidx_in[:, info["off16"] : info["off16"] + B // 16],
                        )
                        msgs = eb.tile([P, SCMAX, P], F16, tag="msgs")
                        nc.gpsimd.dma_gather(
                            msgs[:, 0:sc, :],
                            table[gbase : gbase + gsz, :],
                            gi[:, 0 : B // 16],
                            B,
                            B,
                            P,
                            single_packet=False,
                        )
                        ind = eb.tile([P, SCMAX, P], F16, tag="ind")
                        oc = info["offch"]
                        nc.vector.tensor_tensor(
                            out=ind[:, 0:sc, :],
                            in0=iota16[:, :]
                            .rearrange("p (s d) -> p s d", s=1)
                            .to_broadcast([P, sc, P]),
                            in1=dloc[:, oc : oc + sc]
                            .rearrange("p (s o) -> p s o", o=1)
                            .to_broadcast([P, sc, P]),
                            op=mybir.AluOpType.is_equal,
                        )
                        for k, entries in enumerate(info["cov"]):
                            for t, lo, hi in entries:
                                acc_mm(
                                    t,
                                    msgs[lo:hi, k, 0:width],
                                    ind[lo:hi, k, :],
                                )
                    for ti, t in enumerate(blk):
                        assert mm_done[t] == mm_total[t], (bi, t)
                        evac(t, psums[t])
                    del psums

            # ---- Phase D: layer-1 edges + relu + layer-2 transform ----
            def evac1(t, pa):
                nc.scalar.activation(
                    rt16[:, t * P : (t + 1) * P],
                    pa,
                    mybir.ActivationFunctionType.Relu,
                )
                pg = pst2.tile([P, C], F32, tag="ptx")
                nc.tensor.matmul(
                    pg[:], rt16[:, t * P : (t + 1) * P], w2[:], start=True, stop=True
                )
                nc.scalar.activation(
                    stage2[:, t, 0:C],
                    pg[:],
                    mybir.ActivationFunctionType.Copy,
                    scale=dinv2_nm[:, t : t + 1],
                )

            edge_phase(hfull, H, b1, H, stage, evac1)
            nc.sync.dma_start(
                gsh.rearrange("(t p) d -> p t d", p=P)[:, :, 0:C], stage2[:]
            )

            # ---- Phase E: AllGather layer-2 table ----
            nc.gpsimd.collective_compute(
                "AllGather",
                mybir.AluOpType.bypass,
                ins=[gsh[:]],
                outs=[gfull[:]],
                replica_groups=rgroups,
            )

            # ---- Phase F: layer-2 edges + final transpose/scale ----
            out_stage = stg.tile([P, T, C], F32, tag="ostage")

            def evac2(t, pa):
                sb = io.tile([C, P], F32, tag="ev2")
                nc.vector.tensor_copy(sb[:], pa)
                ptr = pst2.tile([P, C], F32, tag="ptx")
                nc.tensor.transpose(
                    out=ptr[:], in_=sb[:], identity=ident[0:C, 0:C]
                )
                nc.scalar.activation(
                    out_stage[:, t, :],
                    ptr[:],
                    mybir.ActivationFunctionType.Copy,
                    scale=dinv_nm[:, t : t + 1],
                )

            edge_phase(gfull, C, b2, C, stage2, evac2)
            nc.sync.dma_start(out_ext.rearrange("(t p) c -> p t c", p=P)[:], out_stage[:])

    nc.compile()
    return nc


@functools.lru_cache(maxsize=8)
def _build_cached(cfg_key):
    Fdim, H, C, NS, ecnt_t = cfg_key
    return _build((Fdim, H, C, NS, [list(g) for g in ecnt_t]))


# ----------------------------------------------------------------------------
# Host-side sharding / metadata prep
# ----------------------------------------------------------------------------


def _prep(x, edge_index, W1, b1, W2, b2):
    N, Fdim = x.shape
    H = W1.shape[1]
    C = W2.shape[1]
    NS = _round_up(-(-N // NCORE), P)
    T = NS // P
    NPAD = NCORE * NS
    NG = -(-NPAD // GSZ)

    src = np.asarray(edge_index[0], dtype=np.int64)
    dst = np.asarray(edge_index[1], dtype=np.int64)

    deg = np.bincount(dst, minlength=N).astype(np.float32) + 1.0  # + self loop
    deg_pad = np.ones(NPAD, dtype=np.float32)
    deg_pad[:N] = deg

    core = dst // NS
    t_of = (dst % NS) >> 7
    g_of = src >> GSHIFT
    d_of = dst & (P - 1)

    seg_id = (core * NG + g_of) * T + t_of
    cnt = np.bincount(seg_id, minlength=NCORE * NG * T).reshape(NCORE, NG, T)
    ecnt = _round_up(cnt.max(axis=0), 64)  # [NG, T] shared; 64: legal PE base partitions are 0,32,64

    blocks = [list(range(b, min(b + TBSZ, T))) for b in range(0, T, TBSZ)]
    ecnt_l = [[int(v) for v in row] for row in ecnt]
    calls, E_IDX, NCH = _call_layout(ecnt_l, blocks, NG)

    # global idx-stream position of each (g,t) segment
    seg_base = np.zeros((NG, T), dtype=np.int64)
    for bi, blk in enumerate(blocks):
        for g in range(NG):
            info = calls[(bi, g)]
            s = 0
            for t in blk:
                seg_base[g, t] = info["off16"] * 16 + s
                s += ecnt[g, t]

    # rank of each edge inside its (core,g,t) segment
    order = np.argsort(seg_id, kind="stable")
    seg_sorted = seg_id[order]
    starts = np.searchsorted(seg_sorted, np.arange(NCORE * NG * T))
    rank = np.arange(len(order)) - starts[seg_sorted]
    g_sorted = (seg_sorted // T) % NG
    t_sorted = seg_sorted % T
    pos_sorted = seg_base[g_sorted, t_sorted] + rank
    core_sorted = seg_sorted // (NG * T)

    gidx_all = np.zeros((NCORE, E_IDX), dtype=np.int16)
    dloc_all = np.full((NCORE, NCH * P), -1.0, dtype=np.float16)
    # map idx-stream position -> chunk-grid position (call-aligned)
    chunk_pos = np.zeros(max(E_IDX, 1), dtype=np.int64)
    for bi in range(len(blocks)):
        for g in range(NG):
            info = calls[(bi, g)]
            B = info["B"]
            if B == 0:
                continue
            a = info["off16"] * 16
            chunk_pos[a : a + B] = info["offch"] * P + np.arange(B)

    gidx_all[core_sorted, pos_sorted] = (
        src[order] - (g_of[order] << GSHIFT)
    ).astype(np.int16)
    dloc_all[core_sorted, chunk_pos[pos_sorted]] = d_of[order].astype(np.float16)

    x_pad = np.zeros((NPAD, Fdim), dtype=np.float32)
    x_pad[:N] = np.asarray(x, dtype=np.float32)

    in_maps = []
    for c in range(NCORE):
        xT = np.ascontiguousarray(x_pad[c * NS : (c + 1) * NS].T)
        dshard = deg_pad[c * NS : (c + 1) * NS]
        deg_nm = np.ascontiguousarray(dshard.reshape(T, P).T)
        deg_row = dshard.reshape(1, NS)
        flat = gidx_all[c]
        gidx_w = np.tile(
            np.ascontiguousarray(flat.reshape(E_IDX // 16, 16).T), (NCORE, 1)
        )
        dloc_w = np.ascontiguousarray(dloc_all[c].reshape(NCH, P).T)
        in_maps.append(
            {
                "xT": xT,
                "deg_nm": deg_nm,
                "deg_row": deg_row,
                "W1": np.asarray(W1, dtype=np.float32).reshape(Fdim, H),
                "W2": np.asarray(W2, dtype=np.float32).reshape(H, C),
                "b1": np.asarray(b1, dtype=np.float32).reshape(1, H),
                "b2": np.asarray(b2, dtype=np.float32).reshape(1, C),
                "gidx": gidx_w,
                "dloc": dloc_w,
            }
        )

    cfg_key = (Fdim, H, C, NS, tuple(tuple(int(v) for v in row) for row in ecnt))
    return cfg_key, in_maps, N, NS, C


def _run(x, edge_index, W1, b1, W2, b2, trace=False):
    cfg_key, in_maps, N, NS, C = _prep(x, edge_index, W1, b1, W2, b2)
    nc = _build_cached(cfg_key)
    res = run_bass_kernel_spmd(nc, in_maps, list(range(NCORE)), trace=trace)
    shards = [res.results[c]["out_nm"] for c in range(NCORE)]
    out = np.concatenate(shards, axis=0)[:N]
    return np.ascontiguousarray(out, dtype=np.float32), res


def kernel(x, edge_index, W1, b1, W2, b2):
    out, _ = _run(x, edge_index, W1, b1, W2, b2)
    return out
